# revision 1
# baseline (speedup 1.0000x reference)
"""Causal self-attention with RoPE on 8 Trainium2 NeuronCores.

Strategy (tensor-parallel over heads, SPMD-uniform, collective-free):
  - 12 heads -> 8 cores x 2 head slots (4 slots get zero weights).
  - Per core: QKV projection for its 2 heads in [channel, seq] layout;
    RoPE via 3 DVE tensor-tensor ops with sign-folded cos/sin tables;
    causal flash-style attention with scores kept transposed
    (S^T[keys, queries]) so P tiles feed the AV matmul directly; softmax
    denominators via a ones-column in V; per-head normalization; partial
    output projection through the core's slice of w_proj columns.
  - QKV chunk J is interleaved with attention chunk J (causality means
    chunk J only attends keys < 512(J+1)), keeping ScalarE (exp) busy
    from the start.
  - Host sums the 8 partial [C, T] outputs (the "all-reduce") and
    transposes back.  All matmuls run in float32r (TF32-like).
"""

import sys

sys.path.insert(0, "/opt/trn_rl_repo")

import numpy as np

import concourse.bass as bass
import concourse.mybir as mybir
import concourse.tile as tile
from concourse import bacc, bass_utils
from concourse.masks import make_identity

FP32 = mybir.dt.float32
FP32R = mybir.dt.float32r
AF = mybir.ActivationFunctionType
ALU = mybir.AluOpType

T = 4096
C = 768
D = 64
N_HEAD = 12
N_CORES = 8
CHUNK = 512          # query chunk (matmul free dim)
NCHUNK = T // CHUNK  # 8
KT = 128             # key tile
ROPE_BASE = 10000.0

# core -> (head_slot_a, head_slot_b); None = zero slot
HEAD_MAP = [(0, 8), (1, 9), (2, 10), (3, 11),
            (4, None), (5, None), (6, None), (7, None)]

_PROG = None  # cached compiled program


def build_program():
    """Build + compile the per-core Bass program (identical on all cores)."""
    nc = bacc.Bacc("TRN2", target_bir_lowering=False, debug=False,
                   num_devices=N_CORES)

    xT_d = nc.dram_tensor("xT", [C, T], FP32R, kind="ExternalInput").ap()
    wqk_u_d = nc.dram_tensor("wqk_u", [C, 256], FP32R, kind="ExternalInput").ap()
    wqk_w_d = nc.dram_tensor("wqk_w", [C, 256], FP32R, kind="ExternalInput").ap()
    w_v_d = nc.dram_tensor("w_v", [C, 128], FP32R, kind="ExternalInput").ap()
    w_pT_d = nc.dram_tensor("w_projT", [128, C], FP32R, kind="ExternalInput").ap()
    cos_d = nc.dram_tensor("rope_cos", [128, T], FP32, kind="ExternalInput").ap()
    sin_d = nc.dram_tensor("rope_sin", [128, T], FP32, kind="ExternalInput").ap()
    mask_d = nc.dram_tensor("masks", [128, 4 * CHUNK], FP32R, kind="ExternalInput").ap()
    out_d = nc.dram_tensor("outT", [C, T], FP32, kind="ExternalOutput").ap()

    with tile.TileContext(nc) as tc:
        with (
            tc.tile_pool(name="persist", bufs=1) as pers,
            tc.tile_pool(name="xin", bufs=2) as xin,
            tc.tile_pool(name="tmp", bufs=3) as tmps,
            tc.tile_pool(name="ptile", bufs=6) as ptile,
            tc.tile_pool(name="ostage", bufs=3) as ostage,
            tc.tile_pool(name="small", bufs=4) as small,
            tc.tile_pool(name="psUW", bufs=2, space="PSUM") as psUW,  # qkv accumulators
            tc.tile_pool(name="psS", bufs=3, space="PSUM") as psS,    # score tiles
            tc.tile_pool(name="psY", bufs=1, space="PSUM") as psY,    # y accum
            tc.tile_pool(name="psX", bufs=1, space="PSUM") as psX,    # aux (pb/tr) + o
        ):
            # ---- persistent SBUF ----
            wqk_u = pers.tile([128, 6, 256], FP32R)
            wqk_w = pers.tile([128, 6, 256], FP32R)
            w_v = pers.tile([128, 6, 128], FP32R)
            w_pT = pers.tile([128, C], FP32R)

            mask_sb = pers.tile([128, 4 * CHUNK], FP32R)
            QT = pers.tile([128, T], FP32R)   # rows 0-63 head A, 64-127 head B
            KTt = pers.tile([128, T], FP32R)
            V = pers.tile([128, 32, 130], FP32R)  # [key%128, keytile, vA|1|vB|1]
            Y = pers.tile([128, T], FP32R)    # normalized attention out [ych, q]
            ones_sb = pers.tile([128, D], FP32R)
            ident = pers.tile([128, 128], FP32)
            make_identity(nc, ident[:])

            nc.sync.dma_start(wqk_u[:], wqk_u_d.rearrange("(o p) m -> p o m", p=128))
            nc.gpsimd.dma_start(w_v[:], w_v_d.rearrange("(o p) m -> p o m", p=128))
            nc.gpsimd.dma_start(w_pT[:], w_pT_d[:])
            nc.gpsimd.dma_start(mask_sb[:], mask_d[:])
            ones_f32 = pers.tile([128, D], FP32)
            nc.any.memset(ones_f32[:], 1.0)
            # HAM warm-up: keep PE busy during the initial input DMAs so the
            # clock gate reaches 8/8 before the first real matmuls (results
            # discarded; the tiny copy keeps DCE from dropping the chain)
            warm_ps = psS.tile([128, 128], FP32, tag="s")
            for i in range(16):
                nc.tensor.matmul(warm_ps[:], ident[:, 0:128], ident[:, 0:128],
                                 start=True, stop=True)
            warm_sb = small.tile([1, 8], FP32, tag="warm")
            nc.vector.tensor_copy(warm_sb[:], warm_ps[0:1, 0:8])
            nc.vector.tensor_copy(ones_sb[:], ones_f32[:])
            nc.vector.tensor_copy(V[:, :, 64], ones_sb[:, 0:32])
            nc.vector.tensor_copy(V[:, :, 129], ones_sb[:, 0:32])

            def emit_qkv(J):
                cols = slice(J * CHUNK, (J + 1) * CHUNK)
                xt = xin.tile([128, 6, CHUNK], FP32R)
                xT_r = xT_d.rearrange("(o p) n -> p o n", p=128)
                nc.sync.dma_start(xt[:, 0:3, :], xT_r[:, 0:3, cols])
                nc.sync.dma_start(xt[:, 3:6, :], xT_r[:, 3:6, cols])
                if J == 0:
                    nc.sync.dma_start(
                        wqk_w[:], wqk_w_d.rearrange("(o p) m -> p o m", p=128))
                cs_sb = tmps.tile([128, CHUNK], FP32, tag="cs")
                sn_sb = tmps.tile([128, CHUNK], FP32, tag="sn")
                nc.sync.dma_start(cs_sb[:], cos_d[:, cols])
                nc.sync.dma_start(sn_sb[:], sin_d[:, cols])
                for qk, tgt in ((0, QT), (1, KTt)):
                    u_ps = psUW.tile([128, CHUNK], FP32, tag="uw")
                    w_ps = psUW.tile([128, CHUNK], FP32, tag="uw")
                    wcol = slice(qk * 128, qk * 128 + 128)
                    for k in range(6):
                        nc.tensor.matmul(u_ps[:], wqk_u[:, k, wcol], xt[:, k, :],
                                         start=(k == 0), stop=(k == 5))
                    for k in range(6):
                        nc.tensor.matmul(w_ps[:], wqk_w[:, k, wcol], xt[:, k, :],
                                         start=(k == 0), stop=(k == 5))
                    tm = tmps.tile([128, CHUNK], FP32R, tag="ropetmp")
                    nc.vector.tensor_tensor(tgt[:, cols], u_ps[:], cs_sb[:], ALU.mult)
                    nc.vector.tensor_tensor(tm[:], w_ps[:], sn_sb[:], ALU.mult)
                    nc.vector.tensor_tensor(tgt[:, cols], tgt[:, cols], tm[:], ALU.add)
                # v in [ch, seq] (N=512 full-rate), then PE-transpose per key tile
                v_ps = psUW.tile([128, CHUNK], FP32, tag="uw")
                for k in range(6):
                    nc.tensor.matmul(v_ps[:], w_v[:, k, :], xt[:, k, :],
                                     start=(k == 0), stop=(k == 5))
                vT_sb = tmps.tile([128, CHUNK], FP32, tag="vt")
                nc.vector.tensor_copy(vT_sb[:], v_ps[:])
                for s in range(4):
                    kt_idx = 4 * J + s
                    tr_ps = psX.tile([128, 128], FP32, tag="aux")
                    nc.tensor.transpose(tr_ps[:], vT_sb[:, s * 128:(s + 1) * 128], ident)
                    nc.vector.tensor_copy(V[:, kt_idx, 0:64], tr_ps[:, 0:64])
                    nc.vector.tensor_copy(V[:, kt_idx, 65:129], tr_ps[:, 64:128])

            def emit_att(J):
                cols = slice(J * CHUNK, (J + 1) * CHUNK)
                nkt = 4 * J + 4
                for h in range(2):
                    hsl = slice(64 * h, 64 * h + 64)
                    vsl = slice(65 * h, 65 * h + 65)
                    y_ps = psY.tile([65, CHUNK], FP32, tag="y")
                    for t in range(nkt):
                        d = t - 4 * J
                        qlo = max(0, 128 * d)   # cols < qlo have no valid keys in tile t
                        sub = slice(qlo, CHUNK)
                        qsub = slice(J * CHUNK + qlo, (J + 1) * CHUNK)
                        s_ps = psS.tile([128, CHUNK], FP32, tag="s")
                        nc.tensor.matmul(
                            s_ps[:, sub], KTt[hsl, t * KT:(t + 1) * KT], QT[hsl, qsub],
                            start=True, stop=True)
                        p_sb = ptile.tile([128, CHUNK], FP32R, tag="p")
                        nc.scalar.activation(p_sb[:, sub], s_ps[:, sub], AF.Exp, scale=0.125)
                        if d >= 0:
                            # only qq in [qlo, qlo+128) straddles the causal
                            # boundary; columns beyond are fully valid
                            msub = slice(qlo, qlo + KT)
                            nc.vector.tensor_tensor(
                                p_sb[:, msub], p_sb[:, msub],
                                mask_sb[:, d * CHUNK + qlo:d * CHUNK + qlo + KT],
                                ALU.mult)
                        nc.tensor.matmul(
                            y_ps[:, sub], V[:, t, vsl], p_sb[:, sub],
                            start=(t == 0), stop=(t == nkt - 1))
                    rc = small.tile([1, CHUNK], FP32R, tag="rc")
                    with nc.allow_low_precision(reason="f32r recip for softmax denom"):
                        nc.vector.reciprocal(rc[0:1, :], y_ps[64:65, :])
                    rb = small.tile([64, CHUNK], FP32R, tag="rb")
                    nc.gpsimd.partition_broadcast(rb[:], rc[0:1, :])
                    nc.vector.tensor_tensor(Y[hsl, cols], y_ps[0:64, :], rb[:], ALU.mult)
                for m in range(6):
                    # last chunk: transposes are done, so borrow the aux slot
                    # to double-buffer the projection psum
                    otag = "aux" if (J == NCHUNK - 1 and m % 2) else "o"
                    o_ps = psX.tile([128, CHUNK], FP32, tag=otag)
                    nc.tensor.matmul(o_ps[:], w_pT[:, m * 128:(m + 1) * 128],
                                     Y[:, cols], start=True, stop=True)
                    o_sb = ostage.tile([128, CHUNK], FP32, tag="osb")
                    nc.vector.tensor_copy(o_sb[:], o_ps[:])
                    nc.sync.dma_start(out_d[m * 128:(m + 1) * 128, cols], o_sb[:])

            # attention J emitted right after its QKV; later QKV fills PE idle
            for J in range(NCHUNK):
                emit_qkv(J)
                emit_att(J)

    nc.compile()
    return nc


def _rope_tables():
    theta = 1.0 / (ROPE_BASE ** (np.arange(0, D, 2, dtype=np.float32) / D))  # [32]
    freqs = np.arange(T, dtype=np.float32)[None, :] * theta[:, None]  # [32, T]
    cos32 = np.cos(freqs).astype(np.float32)
    sin32 = np.sin(freqs).astype(np.float32)
    cos128 = np.tile(cos32, (4, 1))
    sin128 = np.concatenate([-sin32, sin32, -sin32, sin32], axis=0)
    return cos128, sin128


def _masks():
    m = np.zeros((128, 4 * CHUNK), dtype=np.float32)
    kk = np.arange(128)[:, None]
    qq = np.arange(CHUNK)[None, :]
    for d in range(4):
        m[:, d * CHUNK:(d + 1) * CHUNK] = (128 * d + kk <= qq).astype(np.float32)
    return m


def _swap_halves(w):
    # w: [rows multiple of 64, C]; swap 32-row halves within each 64 block
    r = w.reshape(-1, 2, 32, w.shape[-1])
    return r[:, ::-1].reshape(w.shape)


def make_in_maps(x, w_attn, w_proj):
    xT = np.ascontiguousarray(x.reshape(T, C).T)  # [C, T]
    cos128, sin128 = _rope_tables()
    masks = _masks()
    in_maps = []
    for c in range(N_CORES):
        qk_rows = []   # rows of w_attn for [qA, qB, kA, kB]
        v_rows = []    # [vA, vB]
        p_cols = []    # w_proj columns for [A(64), B(64)]
        sel = HEAD_MAP[c]
        for part_base in (0, C):  # q rows then k rows
            for h in sel:
                if h is None:
                    qk_rows.append(np.zeros((64, C), np.float32))
                else:
                    qk_rows.append(w_attn[part_base + 64 * h: part_base + 64 * h + 64])
        for h in sel:
            if h is None:
                v_rows.append(np.zeros((64, C), np.float32))
                p_cols.append(np.zeros((C, 64), np.float32))
            else:
                v_rows.append(w_attn[2 * C + 64 * h: 2 * C + 64 * h + 64])
                p_cols.append(w_proj[:, 64 * h: 64 * h + 64])
        qk = np.concatenate(qk_rows, axis=0)          # [256, C]
        wqk_u = np.ascontiguousarray(qk.T)            # [C, 256]
        wqk_w = np.ascontiguousarray(_swap_halves(qk).T)
        w_v = np.ascontiguousarray(np.concatenate(v_rows, axis=0).T)  # [C, 128]
        w_pT = np.ascontiguousarray(np.concatenate(p_cols, axis=1).T)  # [128, C]
        in_maps.append({
            "xT": xT, "wqk_u": wqk_u, "wqk_w": wqk_w, "w_v": w_v,
            "w_projT": w_pT, "rope_cos": cos128, "rope_sin": sin128,
            "masks": masks,
        })
    return in_maps


def kernel(x, w_attn, w_proj):
    global _PROG
    x = np.asarray(x, dtype=np.float32)
    w_attn = np.asarray(w_attn, dtype=np.float32)
    w_proj = np.asarray(w_proj, dtype=np.float32)
    if _PROG is None:
        _PROG = build_program()
    nc = _PROG
    in_maps = make_in_maps(x, w_attn, w_proj)
    res = bass_utils.run_bass_kernel_spmd(nc, in_maps, core_ids=list(range(N_CORES)))
    acc = np.zeros((C, T), dtype=np.float64)
    for c in range(N_CORES):
        acc += res.results[c]["outT"].astype(np.float64)
    return np.ascontiguousarray(acc.T.astype(np.float32)).reshape(1, T, C)


if __name__ == "__main__":
    rng = np.random.default_rng(0)
    x = rng.standard_normal((1, T, C)).astype(np.float32)
    wa = (rng.standard_normal((3 * C, C)) * 0.02).astype(np.float32)
    wp = (rng.standard_normal((C, C)) * 0.02).astype(np.float32)
    y = kernel(x, wa, wp)
    print("kernel out", y.shape, y.dtype, float(np.abs(y).max()))



# revision 3
# speedup vs baseline: 16.7508x; 16.7508x over previous
"""Causal self-attention with RoPE on 8 Trainium2 NeuronCores.

Strategy (tensor-parallel over heads, SPMD-uniform, collective-free):
  - 12 heads -> 8 cores x 2 head slots (4 slots get zero weights).
  - Per core: QKV projection for its 2 heads in [channel, seq] layout;
    RoPE via 3 DVE tensor-tensor ops with sign-folded cos/sin tables;
    causal flash-style attention with scores kept transposed
    (S^T[keys, queries]) so P tiles feed the AV matmul directly; softmax
    denominators via a ones-column in V; per-head normalization; partial
    output projection through the core's slice of w_proj columns.
  - QKV chunk J is interleaved with attention chunk J (causality means
    chunk J only attends keys < 512(J+1)), keeping ScalarE (exp) busy
    from the start.
  - Host sums the 8 partial [C, T] outputs (the "all-reduce") and
    transposes back.  All matmuls run in float32r (TF32-like).
"""

import sys

sys.path.insert(0, "/opt/trn_rl_repo")

import numpy as np

import concourse.bass as bass
import concourse.mybir as mybir
import concourse.tile as tile
from concourse import bacc, bass_utils
from concourse.masks import make_identity

FP32 = mybir.dt.float32
FP32R = mybir.dt.float32r
AF = mybir.ActivationFunctionType
ALU = mybir.AluOpType

T = 4096
C = 768
D = 64
N_HEAD = 12
N_CORES = 8
CHUNK = 512          # query chunk (matmul free dim)
NCHUNK = T // CHUNK  # 8
KT = 128             # key tile
ROPE_BASE = 10000.0

# core -> (head_slot_a, head_slot_b); None = zero slot
HEAD_MAP = [(0, 8), (1, 9), (2, 10), (3, 11),
            (4, None), (5, None), (6, None), (7, None)]

_PROG = None  # cached compiled program


def build_program(reps=1):
    """Build + compile the per-core Bass program (identical on all cores).

    ``reps > 1`` wraps the entire kernel body in a hardware loop executing
    it ``reps`` times back-to-back — used by the timing harness to amortize
    the (multi-ms) axon-tunnel dispatch overhead over many on-device
    executions so the per-execution slope isolates true HW time.  Every
    iteration performs the complete kernel, including all input DMA."""
    import contextlib

    nc = bacc.Bacc("TRN2", target_bir_lowering=False, debug=False,
                   num_devices=N_CORES)

    xT_d = nc.dram_tensor("xT", [C, T], FP32R, kind="ExternalInput").ap()
    wqk_u_d = nc.dram_tensor("wqk_u", [C, 256], FP32R, kind="ExternalInput").ap()
    wqk_w_d = nc.dram_tensor("wqk_w", [C, 256], FP32R, kind="ExternalInput").ap()
    w_v_d = nc.dram_tensor("w_v", [C, 128], FP32R, kind="ExternalInput").ap()
    w_pT_d = nc.dram_tensor("w_projT", [128, C], FP32R, kind="ExternalInput").ap()
    cos_d = nc.dram_tensor("rope_cos", [128, T], FP32, kind="ExternalInput").ap()
    sin_d = nc.dram_tensor("rope_sin", [128, T], FP32, kind="ExternalInput").ap()
    mask_d = nc.dram_tensor("masks", [128, 4 * CHUNK], FP32R, kind="ExternalInput").ap()
    out_d = nc.dram_tensor("outT", [C, T], FP32, kind="ExternalOutput").ap()

    with tile.TileContext(nc) as tc:
        with (
            tc.tile_pool(name="persist", bufs=1) as pers,
            tc.tile_pool(name="xin", bufs=2) as xin,
            tc.tile_pool(name="tmp", bufs=3) as tmps,
            tc.tile_pool(name="ptile", bufs=6) as ptile,
            tc.tile_pool(name="ostage", bufs=3) as ostage,
            tc.tile_pool(name="small", bufs=4) as small,
            tc.tile_pool(name="psUW", bufs=2, space="PSUM") as psUW,  # qkv accumulators
            tc.tile_pool(name="psS", bufs=3, space="PSUM") as psS,    # score tiles
            tc.tile_pool(name="psY", bufs=1, space="PSUM") as psY,    # y accum
            tc.tile_pool(name="psX", bufs=1, space="PSUM") as psX,    # aux (pb/tr) + o
            tc.For_i(0, reps) if reps > 1 else contextlib.nullcontext(),
        ):
            # ---- persistent SBUF ----
            wqk_u = pers.tile([128, 6, 256], FP32R)
            wqk_w = pers.tile([128, 6, 256], FP32R)
            w_v = pers.tile([128, 6, 128], FP32R)
            w_pT = pers.tile([128, C], FP32R)

            mask_sb = pers.tile([128, 4 * CHUNK], FP32R)
            QT = pers.tile([128, T], FP32R)   # rows 0-63 head A, 64-127 head B
            KTt = pers.tile([128, T], FP32R)
            V = pers.tile([128, 32, 130], FP32R)  # [key%128, keytile, vA|1|vB|1]
            Y = pers.tile([128, T], FP32R)    # normalized attention out [ych, q]
            ones_sb = pers.tile([128, D], FP32R)
            ident = pers.tile([128, 128], FP32)
            make_identity(nc, ident[:])

            nc.sync.dma_start(wqk_u[:], wqk_u_d.rearrange("(o p) m -> p o m", p=128))
            nc.gpsimd.dma_start(w_v[:], w_v_d.rearrange("(o p) m -> p o m", p=128))
            nc.gpsimd.dma_start(w_pT[:], w_pT_d[:])
            nc.gpsimd.dma_start(mask_sb[:], mask_d[:])
            ones_f32 = pers.tile([128, D], FP32)
            nc.any.memset(ones_f32[:], 1.0)
            # HAM warm-up: keep PE busy during the initial input DMAs so the
            # clock gate reaches 8/8 before the first real matmuls (results
            # discarded; the tiny copy keeps DCE from dropping the chain)
            warm_ps = psS.tile([128, 128], FP32, tag="s")
            for i in range(16):
                nc.tensor.matmul(warm_ps[:], ident[:, 0:128], ident[:, 0:128],
                                 start=True, stop=True)
            warm_sb = small.tile([1, 8], FP32, tag="warm")
            nc.vector.tensor_copy(warm_sb[:], warm_ps[0:1, 0:8])
            nc.vector.tensor_copy(ones_sb[:], ones_f32[:])
            nc.vector.tensor_copy(V[:, :, 64], ones_sb[:, 0:32])
            nc.vector.tensor_copy(V[:, :, 129], ones_sb[:, 0:32])

            def emit_qkv(J):
                cols = slice(J * CHUNK, (J + 1) * CHUNK)
                xt = xin.tile([128, 6, CHUNK], FP32R)
                xT_r = xT_d.rearrange("(o p) n -> p o n", p=128)
                nc.sync.dma_start(xt[:, 0:3, :], xT_r[:, 0:3, cols])
                nc.sync.dma_start(xt[:, 3:6, :], xT_r[:, 3:6, cols])
                if J == 0:
                    nc.sync.dma_start(
                        wqk_w[:], wqk_w_d.rearrange("(o p) m -> p o m", p=128))
                cs_sb = tmps.tile([128, CHUNK], FP32, tag="cs")
                sn_sb = tmps.tile([128, CHUNK], FP32, tag="sn")
                nc.sync.dma_start(cs_sb[:], cos_d[:, cols])
                nc.sync.dma_start(sn_sb[:], sin_d[:, cols])
                for qk, tgt in ((0, QT), (1, KTt)):
                    u_ps = psUW.tile([128, CHUNK], FP32, tag="uw")
                    w_ps = psUW.tile([128, CHUNK], FP32, tag="uw")
                    wcol = slice(qk * 128, qk * 128 + 128)
                    for k in range(6):
                        nc.tensor.matmul(u_ps[:], wqk_u[:, k, wcol], xt[:, k, :],
                                         start=(k == 0), stop=(k == 5))
                    for k in range(6):
                        nc.tensor.matmul(w_ps[:], wqk_w[:, k, wcol], xt[:, k, :],
                                         start=(k == 0), stop=(k == 5))
                    tm = tmps.tile([128, CHUNK], FP32R, tag="ropetmp")
                    nc.vector.tensor_tensor(tgt[:, cols], u_ps[:], cs_sb[:], ALU.mult)
                    nc.vector.tensor_tensor(tm[:], w_ps[:], sn_sb[:], ALU.mult)
                    nc.vector.tensor_tensor(tgt[:, cols], tgt[:, cols], tm[:], ALU.add)
                # v in [ch, seq] (N=512 full-rate), then PE-transpose per key tile
                v_ps = psUW.tile([128, CHUNK], FP32, tag="uw")
                for k in range(6):
                    nc.tensor.matmul(v_ps[:], w_v[:, k, :], xt[:, k, :],
                                     start=(k == 0), stop=(k == 5))
                vT_sb = tmps.tile([128, CHUNK], FP32, tag="vt")
                nc.vector.tensor_copy(vT_sb[:], v_ps[:])
                for s in range(4):
                    kt_idx = 4 * J + s
                    tr_ps = psX.tile([128, 128], FP32, tag="aux")
                    nc.tensor.transpose(tr_ps[:], vT_sb[:, s * 128:(s + 1) * 128], ident)
                    nc.vector.tensor_copy(V[:, kt_idx, 0:64], tr_ps[:, 0:64])
                    nc.vector.tensor_copy(V[:, kt_idx, 65:129], tr_ps[:, 64:128])

            def emit_att(J):
                cols = slice(J * CHUNK, (J + 1) * CHUNK)
                nkt = 4 * J + 4
                for h in range(2):
                    hsl = slice(64 * h, 64 * h + 64)
                    vsl = slice(65 * h, 65 * h + 65)
                    y_ps = psY.tile([65, CHUNK], FP32, tag="y")
                    for t in range(nkt):
                        d = t - 4 * J
                        qlo = max(0, 128 * d)   # cols < qlo have no valid keys in tile t
                        sub = slice(qlo, CHUNK)
                        qsub = slice(J * CHUNK + qlo, (J + 1) * CHUNK)
                        s_ps = psS.tile([128, CHUNK], FP32, tag="s")
                        nc.tensor.matmul(
                            s_ps[:, sub], KTt[hsl, t * KT:(t + 1) * KT], QT[hsl, qsub],
                            start=True, stop=True)
                        p_sb = ptile.tile([128, CHUNK], FP32R, tag="p")
                        nc.scalar.activation(p_sb[:, sub], s_ps[:, sub], AF.Exp, scale=0.125)
                        if d >= 0:
                            # only qq in [qlo, qlo+128) straddles the causal
                            # boundary; columns beyond are fully valid
                            msub = slice(qlo, qlo + KT)
                            nc.vector.tensor_tensor(
                                p_sb[:, msub], p_sb[:, msub],
                                mask_sb[:, d * CHUNK + qlo:d * CHUNK + qlo + KT],
                                ALU.mult)
                        nc.tensor.matmul(
                            y_ps[:, sub], V[:, t, vsl], p_sb[:, sub],
                            start=(t == 0), stop=(t == nkt - 1))
                    rc = small.tile([1, CHUNK], FP32R, tag="rc")
                    with nc.allow_low_precision(reason="f32r recip for softmax denom"):
                        nc.vector.reciprocal(rc[0:1, :], y_ps[64:65, :])
                    rb = small.tile([64, CHUNK], FP32R, tag="rb")
                    nc.gpsimd.partition_broadcast(rb[:], rc[0:1, :])
                    nc.vector.tensor_tensor(Y[hsl, cols], y_ps[0:64, :], rb[:], ALU.mult)
                for m in range(6):
                    # last chunk: transposes are done, so borrow the aux slot
                    # to double-buffer the projection psum
                    otag = "aux" if (J == NCHUNK - 1 and m % 2) else "o"
                    o_ps = psX.tile([128, CHUNK], FP32, tag=otag)
                    nc.tensor.matmul(o_ps[:], w_pT[:, m * 128:(m + 1) * 128],
                                     Y[:, cols], start=True, stop=True)
                    o_sb = ostage.tile([128, CHUNK], FP32, tag="osb")
                    nc.vector.tensor_copy(o_sb[:], o_ps[:])
                    nc.sync.dma_start(out_d[m * 128:(m + 1) * 128, cols], o_sb[:])

            # attention J emitted right after its QKV; later QKV fills PE idle
            for J in range(NCHUNK):
                emit_qkv(J)
                emit_att(J)

    nc.compile()
    return nc


def _rope_tables():
    theta = 1.0 / (ROPE_BASE ** (np.arange(0, D, 2, dtype=np.float32) / D))  # [32]
    freqs = np.arange(T, dtype=np.float32)[None, :] * theta[:, None]  # [32, T]
    cos32 = np.cos(freqs).astype(np.float32)
    sin32 = np.sin(freqs).astype(np.float32)
    cos128 = np.tile(cos32, (4, 1))
    sin128 = np.concatenate([-sin32, sin32, -sin32, sin32], axis=0)
    return cos128, sin128


def _masks():
    m = np.zeros((128, 4 * CHUNK), dtype=np.float32)
    kk = np.arange(128)[:, None]
    qq = np.arange(CHUNK)[None, :]
    for d in range(4):
        m[:, d * CHUNK:(d + 1) * CHUNK] = (128 * d + kk <= qq).astype(np.float32)
    return m


def _swap_halves(w):
    # w: [rows multiple of 64, C]; swap 32-row halves within each 64 block
    r = w.reshape(-1, 2, 32, w.shape[-1])
    return r[:, ::-1].reshape(w.shape)


def make_in_maps(x, w_attn, w_proj):
    xT = np.ascontiguousarray(x.reshape(T, C).T)  # [C, T]
    cos128, sin128 = _rope_tables()
    masks = _masks()
    in_maps = []
    for c in range(N_CORES):
        qk_rows = []   # rows of w_attn for [qA, qB, kA, kB]
        v_rows = []    # [vA, vB]
        p_cols = []    # w_proj columns for [A(64), B(64)]
        sel = HEAD_MAP[c]
        for part_base in (0, C):  # q rows then k rows
            for h in sel:
                if h is None:
                    qk_rows.append(np.zeros((64, C), np.float32))
                else:
                    qk_rows.append(w_attn[part_base + 64 * h: part_base + 64 * h + 64])
        for h in sel:
            if h is None:
                v_rows.append(np.zeros((64, C), np.float32))
                p_cols.append(np.zeros((C, 64), np.float32))
            else:
                v_rows.append(w_attn[2 * C + 64 * h: 2 * C + 64 * h + 64])
                p_cols.append(w_proj[:, 64 * h: 64 * h + 64])
        qk = np.concatenate(qk_rows, axis=0)          # [256, C]
        wqk_u = np.ascontiguousarray(qk.T)            # [C, 256]
        wqk_w = np.ascontiguousarray(_swap_halves(qk).T)
        w_v = np.ascontiguousarray(np.concatenate(v_rows, axis=0).T)  # [C, 128]
        w_pT = np.ascontiguousarray(np.concatenate(p_cols, axis=1).T)  # [128, C]
        in_maps.append({
            "xT": xT, "wqk_u": wqk_u, "wqk_w": wqk_w, "w_v": w_v,
            "w_projT": w_pT, "rope_cos": cos128, "rope_sin": sin128,
            "masks": masks,
        })
    return in_maps


def kernel(x, w_attn, w_proj):
    global _PROG
    x = np.asarray(x, dtype=np.float32)
    w_attn = np.asarray(w_attn, dtype=np.float32)
    w_proj = np.asarray(w_proj, dtype=np.float32)
    if _PROG is None:
        _PROG = build_program()
    nc = _PROG
    in_maps = make_in_maps(x, w_attn, w_proj)
    res = bass_utils.run_bass_kernel_spmd(nc, in_maps, core_ids=list(range(N_CORES)))
    acc = np.zeros((C, T), dtype=np.float64)
    for c in range(N_CORES):
        acc += res.results[c]["outT"].astype(np.float64)
    return np.ascontiguousarray(acc.T.astype(np.float32)).reshape(1, T, C)


if __name__ == "__main__":
    rng = np.random.default_rng(0)
    x = rng.standard_normal((1, T, C)).astype(np.float32)
    wa = (rng.standard_normal((3 * C, C)) * 0.02).astype(np.float32)
    wp = (rng.standard_normal((C, C)) * 0.02).astype(np.float32)
    y = kernel(x, wa, wp)
    print("kernel out", y.shape, y.dtype, float(np.abs(y).max()))



# revision 22
# speedup vs baseline: 20.8883x; 1.2470x over previous
"""Causal self-attention with RoPE on 8 Trainium2 NeuronCores.

Strategy (tensor-parallel over heads, SPMD-uniform, collective-free):
  - 12 heads -> 8 cores x 2 head slots (4 slots get zero weights).
  - Per core: QKV projection for its 2 heads in [channel, seq] layout via
    fp8e4 DoubleRow matmuls (weights prescaled x64 on host, score scale
    folded into the exp); RoPE via 3 DVE tensor-tensor ops with
    sign-folded cos/sin tables (the duplicated u/w projection is cheap in
    fp8); V computed directly in [seq, channel] layout with x as the
    stationary operand (no PE transposes), stored fp8 with interleaved
    ones columns for the softmax denominators.
  - Causal flash-style attention with scores kept transposed
    (S^T[keys, queries]): full key tiles are processed in PAIRS - two
    S matmuls into one 2-bank PSUM tile, ONE exp over 1024 columns
    emitting fp8, one DoubleRow AV matmul contracting 256 keys; the 4
    diagonal tiles use the single-tile path with a pre-exp additive
    causal mask (-1e9) on the triangular strip.
  - Per-head normalization via the ones-column denominators; partial
    output projection (fp32r) through the core's slice of w_proj
    columns; bf16 partial outputs. Host sums the 8 partials in f64.
  - QKV chunk J is interleaved with attention chunk J, keeping ScalarE
    (exp) busy from the start.
"""

import sys

sys.path.insert(0, "/opt/trn_rl_repo")

import numpy as np

import concourse.bass as bass
import concourse.mybir as mybir
import concourse.tile as tile
from concourse import bacc, bass_utils
from concourse.masks import make_identity

FP32 = mybir.dt.float32
FP32R = mybir.dt.float32r
FP8 = mybir.dt.float8e4
BF16 = mybir.dt.bfloat16
AF = mybir.ActivationFunctionType
ALU = mybir.AluOpType
DR = mybir.MatmulPerfMode.DoubleRow

T = 4096
C = 768
D = 64
N_HEAD = 12
N_CORES = 8
CHUNK = 512          # query chunk (matmul free dim)
NCHUNK = T // CHUNK  # 8
KT = 128             # key tile
ROPE_BASE = 10000.0
W_SCALE = 64.0       # host prescale on fp8 qkv weights (subnormal avoidance)
EXP_SCALE = 0.125 / (W_SCALE * W_SCALE)  # = 2**-15, folds 1/sqrt(D) + descale

# core -> (head_slot_a, head_slot_b); None = zero slot
HEAD_MAP = [(0, 8), (1, 9), (2, 10), (3, 11),
            (4, None), (5, None), (6, None), (7, None)]

_PROG = None  # cached compiled program


def build_program(reps=1):
    """Build + compile the per-core Bass program (identical on all cores).

    ``reps > 1`` wraps the entire kernel body in a hardware loop executing
    it ``reps`` times back-to-back - used by the timing harness to amortize
    the (multi-ms) axon-tunnel dispatch overhead over many on-device
    executions so the per-execution slope isolates true HW time.  Every
    iteration performs the complete kernel, including all input DMA."""
    import contextlib

    nc = bacc.Bacc("TRN2", target_bir_lowering=False, debug=False,
                   num_devices=N_CORES)

    xT_d = nc.dram_tensor("xT", [C, T], BF16, kind="ExternalInput").ap()
    wqk_u_d = nc.dram_tensor("wqk_u", [C, 256], BF16, kind="ExternalInput").ap()
    wqk_w_d = nc.dram_tensor("wqk_w", [C, 256], BF16, kind="ExternalInput").ap()
    w_v_d = nc.dram_tensor("w_v", [C, 128], BF16, kind="ExternalInput").ap()
    w_pT_d = nc.dram_tensor("w_projT", [128, C], FP32R, kind="ExternalInput").ap()
    cos_d = nc.dram_tensor("rope_cos", [128, T], FP32, kind="ExternalInput").ap()
    sin_d = nc.dram_tensor("rope_sin", [128, T], FP32, kind="ExternalInput").ap()
    mask_d = nc.dram_tensor("maskneg", [128, KT], FP32, kind="ExternalInput").ap()
    out_d = nc.dram_tensor("outT", [C, T], BF16, kind="ExternalOutput").ap()

    with tile.TileContext(nc) as tc:
        with (
            tc.tile_pool(name="persist", bufs=1) as pers,
            tc.tile_pool(name="xin", bufs=2) as xin,
            tc.tile_pool(name="tmp", bufs=3) as tmps,
            tc.tile_pool(name="ptile", bufs=4) as ptile,
            tc.tile_pool(name="ostage", bufs=3) as ostage,
            tc.tile_pool(name="small", bufs=4) as small,
            tc.tile_pool(name="psUW", bufs=2, space="PSUM") as psUW,  # u/w/v accums
            tc.tile_pool(name="psS", bufs=2, space="PSUM") as psS,    # score pairs + proj
            tc.tile_pool(name="psY", bufs=2, space="PSUM") as psY,    # y accum
            tc.For_i(0, reps) if reps > 1 else contextlib.nullcontext(),
        ):
            # ---- persistent SBUF ----
            wqk_u = pers.tile([128, 6, 256], BF16)
            wqk_w = pers.tile([128, 6, 256], BF16)
            w_v = pers.tile([128, 6, 128], BF16)
            w_pT = pers.tile([128, C], FP32R)

            mask_sb = pers.tile([128, KT], FP32)
            QT = pers.tile([128, T], FP32R)   # rows 0-63 head A, 64-127 head B
            KTt = pers.tile([128, T], FP32R)
            V = pers.tile([128, 32, 2, 96], FP8)   # [key%128, keytile, head, v|1|pad]
            Vb = pers.tile([128, 32, 2, 96], BF16)  # bf16 twin for diagonal AV
            # DoubleRow stationary free/2 must be a multiple of 32 -> pad
            # each head's block to 96 (cols 65:96 zeroed, never normalized)
            Y = pers.tile([128, T], FP32R)    # normalized attention out [ych, q]
            ones_sb = pers.tile([128, D], FP8)
            ones_bf = pers.tile([128, D], BF16)
            ident = pers.tile([128, 128], FP32)

            # issue all initial DMAs before any engine work queues up
            nc.sync.dma_start(wqk_u[:], wqk_u_d.rearrange("(o p) m -> p o m", p=128))
            nc.gpsimd.dma_start(w_v[:], w_v_d.rearrange("(o p) m -> p o m", p=128))
            nc.gpsimd.dma_start(w_pT[:], w_pT_d[:])
            nc.gpsimd.dma_start(mask_sb[:], mask_d[:])
            ones_f32 = pers.tile([128, D], FP32)
            nc.any.memset(ones_f32[:], 1.0)
            # preload the Exp activation table off the critical path
            warm_act = small.tile([1, 8], FP32, tag="wact")
            nc.scalar.activation(warm_act[:], ones_f32[0:1, 0:8], AF.Exp)
            make_identity(nc, ident[:])
            # HAM warm-up: keep PE busy during the initial input DMAs so the
            # clock gate reaches 8/8 before the first real matmuls (results
            # discarded; the tiny copy keeps DCE from dropping the chain)
            warm_ps = psS.tile([128, 2, CHUNK], FP32, tag="s")
            for i in range(8):
                nc.tensor.matmul(warm_ps[0:64, 0, 0:64], ident[:, 0:64],
                                 ident[:, 0:64], start=True, stop=True)
            warm_sb = small.tile([1, 8], FP32, tag="warm")
            nc.vector.tensor_copy(warm_sb[:], warm_ps[0:1, 0, 0:8])
            nc.vector.tensor_copy(ones_sb[:], ones_f32[:])
            nc.vector.tensor_copy(ones_bf[:], ones_f32[:])
            nc.vector.tensor_copy(V[:, :, 0, 64], ones_sb[:, 0:32])
            nc.vector.tensor_copy(V[:, :, 1, 64], ones_sb[:, 0:32])
            nc.vector.tensor_copy(Vb[:, :, 0, 64], ones_bf[:, 0:32])
            nc.vector.tensor_copy(Vb[:, :, 1, 64], ones_bf[:, 0:32])
            nc.gpsimd.memset(V[:, :, :, 65:96], 0.0)
            nc.gpsimd.memset(Vb[:, :, :, 65:96], 0.0)

            def emit_qkv(J):
                cols = slice(J * CHUNK, (J + 1) * CHUNK)
                xt = xin.tile([128, 6, CHUNK], BF16)
                xT_r = xT_d.rearrange("(o p) n -> p o n", p=128)
                nc.sync.dma_start(xt[:, 0:3, :], xT_r[:, 0:3, cols])
                nc.sync.dma_start(xt[:, 3:6, :], xT_r[:, 3:6, cols])
                if J == 0:
                    nc.sync.dma_start(
                        wqk_w[:], wqk_w_d.rearrange("(o p) m -> p o m", p=128))
                cs_sb = tmps.tile([128, CHUNK], FP32, tag="cs")
                sn_sb = tmps.tile([128, CHUNK], FP32, tag="sn")
                # J=0's tables are on the critical startup path: the scalar
                # HWDGE queue is idle then; steady-state issues ride gpsimd
                dma_eng = nc.scalar if J == 0 else nc.gpsimd
                dma_eng.dma_start(cs_sb[:], cos_d[:, cols])
                dma_eng.dma_start(sn_sb[:], sin_d[:, cols])
                for qk, tgt in ((0, QT), (1, KTt)):
                    u_ps = psUW.tile([128, CHUNK], FP32, tag="uw")
                    w_ps = psUW.tile([128, CHUNK], FP32, tag="uw")
                    wcol = slice(qk * 128, qk * 128 + 128)
                    for k in range(6):
                        nc.tensor.matmul(u_ps[:], wqk_u[:, k, wcol], xt[:, k, :],
                                         start=(k == 0), stop=(k == 5))
                    for k in range(6):
                        nc.tensor.matmul(w_ps[:], wqk_w[:, k, wcol], xt[:, k, :],
                                         start=(k == 0), stop=(k == 5))
                    tm = tmps.tile([128, CHUNK], FP32R, tag="ropetmp")
                    nc.vector.tensor_tensor(tgt[:, cols], u_ps[:], cs_sb[:], ALU.mult)
                    nc.vector.tensor_tensor(tm[:], w_ps[:], sn_sb[:], ALU.mult)
                    nc.vector.tensor_tensor(tgt[:, cols], tgt[:, cols], tm[:], ALU.add)
                # v directly in [seq, ch] layout: x slice as the stationary
                v_ps = psUW.tile([128, CHUNK], FP32, tag="uw")
                for s in range(4):
                    ssl = slice(s * 128, (s + 1) * 128)
                    for k in range(6):
                        nc.tensor.matmul(v_ps[:, ssl], xt[:, k, ssl],
                                         w_v[:, k, :],
                                         start=(k == 0), stop=(k == 5))
                # one copy per twin drops the 4x[seq128, vA64|vB64] quarters
                # into the [keytile, head, 96] layout, skipping ones/pad
                nc.vector.tensor_copy(V[:, 4 * J:4 * J + 4, :, 0:64], v_ps[:])
                nc.vector.tensor_copy(Vb[:, 4 * J:4 * J + 4, :, 0:64], v_ps[:])

            def emit_proj_m(J, m, on_act=False):
                # output projection m-tile pair for chunk J, through the
                # score pool (deferred into chunk J+1's attention so the
                # PSUM-bank handoff and the DVE copies stay off the
                # QKV->attention critical path); the final chunk's copies
                # ride the then-idle ACT engine instead of DVE
                cols = slice(J * CHUNK, (J + 1) * CHUNK)
                o_ps = psS.tile([128, 2, CHUNK], FP32, tag="s")
                for u in range(2):
                    mm = 2 * m + u
                    nc.tensor.matmul(o_ps[:, u, :],
                                     w_pT[:, mm * 128:(mm + 1) * 128],
                                     Y[:, cols], start=True, stop=True)
                o_sb = ostage.tile([128, 2, CHUNK], BF16, tag="osb")
                if on_act:
                    nc.scalar.copy(o_sb[:], o_ps[:])
                else:
                    nc.vector.tensor_copy(o_sb[:], o_ps[:])
                nc.sync.dma_start(
                    out_d.rearrange("(a p) n -> p a n", p=128)[:, 2 * m:2 * m + 2, cols],
                    o_sb[:])

            def emit_pair(J, h, k, y_ps, first, last):
                cols = slice(J * CHUNK, (J + 1) * CHUNK)
                hsl = slice(64 * h, 64 * h + 64)
                s2 = psS.tile([128, 2, CHUNK], FP32, tag="s")
                for u in range(2):
                    t = 2 * k + u
                    nc.tensor.matmul(
                        s2[:, u, :], KTt[hsl, t * KT:(t + 1) * KT],
                        QT[hsl, cols], start=True, stop=True)
                p2 = ptile.tile([128, 2, CHUNK], FP8, tag="p")
                nc.scalar.activation(p2[:], s2[:], AF.Exp, scale=EXP_SCALE)
                nc.tensor.matmul(
                    y_ps[:], V[:, 2 * k:2 * k + 2, h, :], p2[:],
                    start=first, stop=last, perf_mode=DR)

            def emit_diag(J, h, d, y_ps, first, last):
                hsl = slice(64 * h, 64 * h + 64)
                t = 4 * J + d
                qlo = 128 * d
                sub = slice(qlo, CHUNK)
                qsub = slice(J * CHUNK + qlo, (J + 1) * CHUNK)
                s1 = psS.tile([128, 2, CHUNK], FP32, tag="s")
                nc.tensor.matmul(
                    s1[:, 0, sub], KTt[hsl, t * KT:(t + 1) * KT],
                    QT[hsl, qsub], start=True, stop=True)
                nc.vector.tensor_tensor(
                    s1[:, 0, qlo:qlo + KT], s1[:, 0, qlo:qlo + KT],
                    mask_sb[:], ALU.add)
                p1 = ptile.tile([128, CHUNK], BF16, tag="p1")
                nc.scalar.activation(p1[:, sub], s1[:, 0, sub], AF.Exp,
                                     scale=EXP_SCALE)
                nc.tensor.matmul(
                    y_ps[:, sub], Vb[:, t, h, :], p1[:, sub],
                    start=first, stop=last)

            def emit_norm(J, h, y_ps):
                cols = slice(J * CHUNK, (J + 1) * CHUNK)
                hsl = slice(64 * h, 64 * h + 64)
                rc = small.tile([1, CHUNK], FP32R, tag="rc")
                with nc.allow_low_precision(reason="f32r recip for softmax denom"):
                    nc.vector.reciprocal(rc[0:1, :], y_ps[64:65, :])
                rb = small.tile([64, CHUNK], FP32R, tag="rb")
                nc.gpsimd.partition_broadcast(rb[:], rc[0:1, :])
                nc.vector.tensor_tensor(Y[hsl, cols], y_ps[0:64, :], rb[:], ALU.mult)

            def att_items(J):
                # interleave the 4 diagonal (masked) chains among the pair
                # chains so pair exps fill the S->mask->exp latency bubbles;
                # keep the first two and last slots pair-only
                pairs = [("p", k) for k in range(2 * J)]
                diags = [("d", d) for d in range(4)]
                if 2 * J >= 7:
                    items = pairs[:2]
                    rest = pairs[2:]
                    for i, dd in enumerate(diags):
                        items.append(dd)
                        if i < len(rest):
                            items.append(rest[i])
                    items.extend(rest[4:])
                else:
                    items = diags + pairs
                return items

            def emit_item(J, h, it, y_ps, first, last):
                if it[0] == "p":
                    emit_pair(J, h, it[1], y_ps, first, last)
                else:
                    emit_diag(J, h, it[1], y_ps, first, last)

            # Emission order per chunk J (steady state):
            #   h0: interleaved pairs+diagonals, then normalize (pair exps
            #       have no DVE dependency, so they absorb the previous
            #       chunk's deferred proj copies on DVE)
            #   h1: all but the last HOLD items
            #   qkv(J+1)  (PE+DVE work runs while ACT chews the backlog)
            #   h1: last HOLD items with chunk J-1's proj m-pairs woven in
            #       (each proj copy overlaps a held exp), then normalize
            HOLD = 6
            emit_qkv(0)
            for J in range(NCHUNK):
                items = att_items(J)
                n_it = len(items)
                y0 = psY.tile([96, CHUNK], FP32, tag="y")
                for i, it in enumerate(items):
                    emit_item(J, 0, it, y0, i == 0, i == n_it - 1)
                emit_norm(J, 0, y0)
                y1 = psY.tile([96, CHUNK], FP32, tag="y")
                held = min(HOLD, n_it - 2)
                split = n_it - held
                for i, it in enumerate(items[:split]):
                    emit_item(J, 1, it, y1, i == 0, i == n_it - 1)
                if J + 1 < NCHUNK:
                    emit_qkv(J + 1)
                projm = list(range(3)) if J > 0 else []
                for i, it in enumerate(items[split:]):
                    emit_item(J, 1, it, y1, split + i == 0,
                              split + i == n_it - 1)
                    if i % 2 == 1 and projm:
                        emit_proj_m(J - 1, projm.pop(0))
                emit_norm(J, 1, y1)
                for m in projm:
                    emit_proj_m(J - 1, m)
            for m in range(3):
                emit_proj_m(NCHUNK - 1, m, on_act=True)

    nc.compile()
    return nc


def _rope_tables():
    theta = 1.0 / (ROPE_BASE ** (np.arange(0, D, 2, dtype=np.float32) / D))  # [32]
    freqs = np.arange(T, dtype=np.float32)[None, :] * theta[:, None]  # [32, T]
    cos32 = np.cos(freqs).astype(np.float32)
    sin32 = np.sin(freqs).astype(np.float32)
    cos128 = np.tile(cos32, (4, 1))
    sin128 = np.concatenate([-sin32, sin32, -sin32, sin32], axis=0)
    return cos128, sin128


def _masks():
    # additive causal mask for the diagonal 128-strip: key row kk may only
    # attend query column j >= kk (strip-local coords are d-independent)
    kk = np.arange(128)[:, None]
    jj = np.arange(KT)[None, :]
    return np.where(kk > jj, np.float32(-1e9), np.float32(0.0))


def _swap_halves(w):
    # w: [rows multiple of 64, C]; swap 32-row halves within each 64 block
    r = w.reshape(-1, 2, 32, w.shape[-1])
    return r[:, ::-1].reshape(w.shape)


def make_in_maps(x, w_attn, w_proj):
    bf = mybir.dt.np(BF16)
    xT = np.ascontiguousarray(x.reshape(T, C).T).astype(bf)  # [C, T]
    cos128, sin128 = _rope_tables()
    masks = _masks()
    in_maps = []
    for c in range(N_CORES):
        qk_rows = []   # rows of w_attn for [qA, qB, kA, kB]
        v_rows = []    # [vA, vB]
        p_cols = []    # w_proj columns for [A(64), B(64)]
        sel = HEAD_MAP[c]
        for part_base in (0, C):  # q rows then k rows
            for h in sel:
                if h is None:
                    qk_rows.append(np.zeros((64, C), np.float32))
                else:
                    qk_rows.append(w_attn[part_base + 64 * h: part_base + 64 * h + 64])
        for h in sel:
            if h is None:
                v_rows.append(np.zeros((64, C), np.float32))
                p_cols.append(np.zeros((C, 64), np.float32))
            else:
                v_rows.append(w_attn[2 * C + 64 * h: 2 * C + 64 * h + 64])
                p_cols.append(w_proj[:, 64 * h: 64 * h + 64])
        qk = np.concatenate(qk_rows, axis=0) * W_SCALE      # [256, C]
        wqk_u = np.ascontiguousarray(qk.T).astype(bf)       # [C, 256]
        wqk_w = np.ascontiguousarray(_swap_halves(qk).T).astype(bf)
        w_v = np.ascontiguousarray(
            (np.concatenate(v_rows, axis=0) * W_SCALE).T).astype(bf)  # [C, 128]
        w_pT = np.ascontiguousarray(
            np.concatenate(p_cols, axis=1).T / W_SCALE)     # [128, C]
        in_maps.append({
            "xT": xT, "wqk_u": wqk_u, "wqk_w": wqk_w, "w_v": w_v,
            "w_projT": w_pT.astype(np.float32), "rope_cos": cos128,
            "rope_sin": sin128, "maskneg": masks,
        })
    return in_maps


def kernel(x, w_attn, w_proj):
    global _PROG
    x = np.asarray(x, dtype=np.float32)
    w_attn = np.asarray(w_attn, dtype=np.float32)
    w_proj = np.asarray(w_proj, dtype=np.float32)
    if _PROG is None:
        _PROG = build_program()
    nc = _PROG
    in_maps = make_in_maps(x, w_attn, w_proj)
    res = bass_utils.run_bass_kernel_spmd(nc, in_maps, core_ids=list(range(N_CORES)))
    acc = np.zeros((C, T), dtype=np.float64)
    for c in range(N_CORES):
        acc += res.results[c]["outT"].astype(np.float64)
    return np.ascontiguousarray(acc.T.astype(np.float32)).reshape(1, T, C)


if __name__ == "__main__":
    rng = np.random.default_rng(0)
    x = rng.standard_normal((1, T, C)).astype(np.float32)
    wa = (rng.standard_normal((3 * C, C)) * 0.02).astype(np.float32)
    wp = (rng.standard_normal((C, C)) * 0.02).astype(np.float32)
    y = kernel(x, wa, wp)
    print("kernel out", y.shape, y.dtype, float(np.abs(y).max()))
